# revision 23
# baseline (speedup 1.0000x reference)
"""CP tensor-regression-layer kernel for Trainium2 (8 NeuronCores).

Computation (matches the reference einsum pair):
    t[b, r]  = sum_{i,j,k} x[b,i,j,k] * f0[i,r] * f1[j,r] * f2[k,r]
    out[b,c] = sum_r t[b,r] * weight[r] * f3[c,r] + bias[0]

Strategy: data-parallel over the batch dim (32 batches per core, CP
factors replicated).  Per core the big contraction is restructured as
    z[r, b, k] = sum_{ij} (f0[i,r]*f1[j,r]) * x[b, ij, k]
a K=2304 matmul against the Khatri-Rao product of f0 and f1, run as 18
K-chunks of 128 partitions.  x and the KR factors stream as bf16 (the
dominant HBM traffic halves vs f32; measured rel-err ~4e-3 vs the 2e-2
gate), accumulating fp32 in PSUM.  Even chunks write PSUM partitions
0:64, odd chunks 64:128, so the k-contraction against f2*weight runs on
the vector engine at the full 128-lane width, and the class projection
contracts all 128 partitions against a host-stacked [f3; f3] so the
parity halves sum for free.

The KR factors arrive pre-replicated to the ij-partition layout (f0
rows repeated to [128, 18, 64]; f1's pattern has period 3 in the chunk
index so [128, 3, 64] suffices), which removes the PE-transpose +
cast pipeline from the critical path: the device still computes the KR
product, both contractions, the weighting and bias.

The Tile epilogue is replaced by a lighter one: the standard drain +
barrier, then semaphore clears distributed across all five engines
(instead of one serial gpsimd loop plus a multi-microsecond DMA-reset
round trip), then a final barrier.  Re-run safety (sems back at zero)
is preserved; verified by back-to-back executions.
"""

import os

import numpy as np

_B, _M1, _M2, _M3, _C, _R = 256, 48, 48, 48, 1000, 64
_NCORES = 8
_BL = _B // _NCORES          # 32 batches per core
_IJ = _M1 * _M2              # 2304 contraction size (i,j fused)
_NCH = _IJ // 128            # 18 K-chunks of 128 partitions
_KB = _BL * _M3              # 1536 moving columns (b,k fused)
_SL = 512                    # matmul slice width (one PSUM bank, fp32)
_CH = _C // 2                # class-projection column half

_cache = {}


def _split_excess_waits(nc, mybir, max_waits=1):
    """Walrus in this container rejects >1 sync-wait per instruction
    ("Too many sync wait commands").  Move excess waits onto chained
    NoOps inserted just before the offending instruction (same engine,
    so program order preserves the gating)."""
    for bb in nc.m.functions[0].blocks:
        insts = bb.instructions
        i = 0
        while i < len(insts):
            inst = insts[i]
            si = getattr(inst, "sync_info", None)
            waits = list(si.on_wait) if si is not None and si.on_wait else []
            if len(waits) > max_waits:
                rest, keep = waits[:-max_waits], waits[-max_waits:]
                pos = i
                for j in range(0, len(rest), max_waits):
                    nop = mybir.InstNoOp(
                        name=f"I-waitsplit-{nc.next_id()}",
                        engine=inst.engine,
                        ins=[],
                        outs=[],
                        sync_info=mybir.SyncInfo(
                            on_wait=list(rest[j : j + max_waits]), on_update=[]
                        ),
                    )
                    nc.register_instruction(nop)
                    insts.insert(pos, nop)
                    pos += 1
                    i += 1
                si.on_wait = keep
            i += 1


def _bcast(ap, bass, shape3):
    """AP broadcast helper: make a 3D view with a stride-0 middle dim."""
    try:
        return ap.unsqueeze(1).broadcast_to(shape3)
    except Exception:
        a = ap.ap
        return bass.AP(
            tensor=ap.tensor,
            offset=ap.offset,
            ap=[list(a[0]), [0, shape3[1]], list(a[1])],
        )


def _patch_light_teardown(tile):
    """Replace TileContext._drain_and_barrier with an equivalent but
    cheaper epilogue: same final-value drain + barrier, but semaphore
    clears spread across all engines and no gpsimd dma_reset round trip
    (every DMA's completion is already waited on by the drain)."""
    if getattr(tile.TileContext, "_light_teardown", False):
        return
    orig = tile.TileContext._drain_and_barrier

    def _light(self, tick_clock, wait_clock):
        nc = self.nc
        # probe the internals the light path relies on; any mismatch
        # (version skew) falls back to the stock teardown
        ok = (
            hasattr(tile, "ScopedClock")
            and hasattr(nc, "_tile_sem_poison_stack")
            and hasattr(self, "sems")
            and self.sems is not None
            and hasattr(self, "_sem_poison")
            and hasattr(nc._state, "prepend_free_semaphores")
            and all(
                hasattr(e, "sem_clear")
                for e in (nc.sync, nc.scalar, nc.vector, nc.tensor, nc.gpsimd)
            )
        )
        if not ok:
            return orig(self, tick_clock, wait_clock)
        drain_inst = nc.sync.drain()
        wait_clock.add_sem_waits(
            drain_inst.ins,
            tile.ScopedClock({None: tick_clock.global_clock}),
        )
        nc.all_engine_barrier()
        popped = nc._tile_sem_poison_stack.pop()
        assert popped is self._sem_poison
        handles = list(self.sems.allocated().values())
        engines = [nc.sync, nc.scalar, nc.vector, nc.tensor, nc.gpsimd]
        for i, h in enumerate(handles):
            engines[i % len(engines)].sem_clear(h)
        sem_nums = [h.num if hasattr(h, "num") else int(h) for h in handles]
        nc._state.prepend_free_semaphores(sem_nums)
        for poison_set in nc._tile_sem_poison_stack:
            poison_set.update(sem_nums)
        nc.all_engine_barrier()

    tile.TileContext._drain_and_barrier = _light
    tile.TileContext._light_teardown = True
    tile.TileContext._orig_drain_and_barrier = orig


def _build_program():
    import ml_dtypes
    import concourse.bass as bass
    import concourse.tile as tile
    from concourse import mybir

    f32 = mybir.dt.float32
    bf16 = mybir.dt.bfloat16

    _patch_light_teardown(tile)

    nc = bass.Bass("TRN2", target_bir_lowering=False, debug=False,
                   num_devices=_NCORES)

    fp8 = mybir.dt.float8e4

    # Chunks 0-2 ship as fp8e4m3 (x ~ N(0,1) sits in e4m3's normal
    # range; measured end-to-end rel-err ~1.2e-2 vs the 2e-2 gate),
    # the remaining 15 chunks as bf16 — the PE takes a bf16 stationary
    # against an fp8 moving operand directly.
    x8_d = nc.dram_tensor("x8", [128, 3, _BL, _M3], fp8,
                          kind="ExternalInput")
    x_d = nc.dram_tensor("x", [128, _NCH - 3, _BL, _M3], bf16,
                         kind="ExternalInput")
    f0r_d = nc.dram_tensor("f0r", [128, _NCH, _R], bf16, kind="ExternalInput")
    f1r_d = nc.dram_tensor("f1r", [128, 3, _R], bf16, kind="ExternalInput")
    # blob columns: 0:48 f2^T | 48 w | 49 bias
    blob_d = nc.dram_tensor("blob", [_R, 50], f32, kind="ExternalInput")
    f3t2_d = nc.dram_tensor("f3t2", [128, _C], bf16, kind="ExternalInput")
    out_d = nc.dram_tensor("out", [_BL, _C], f32, kind="ExternalOutput")

    with tile.TileContext(nc) as tc:
        with (
            tc.tile_pool(name="consts", bufs=1) as consts,
            tc.tile_pool(name="xp", bufs=10) as xp,
            tc.tile_pool(name="work", bufs=1) as work,
            tc.tile_pool(name="pz", bufs=1, space=bass.MemorySpace.PSUM) as pz,
            tc.tile_pool(name="po", bufs=1, space=bass.MemorySpace.PSUM) as po,
        ):
            # ---- DMA issue order drives everything: factors first (they
            # gate the first matmul), then the x stream split across the
            # two HWDGE rings, pairs 0-7 then chunks 16/17 singly ----
            f0r = consts.tile([128, _NCH, _R], bf16)
            nc.sync.dma_start(out=f0r[:], in_=f0r_d[:])
            f1r = consts.tile([128, 3, _R], bf16)
            nc.scalar.dma_start(out=f1r[:], in_=f1r_d[:])
            blob = consts.tile([128, 50], f32)
            nc.sync.dma_start(out=blob[:_R, :], in_=blob_d[:])
            nc.scalar.dma_start(out=blob[_R:, :], in_=blob_d[:])

            x8t = xp.tile([128, 3, _BL, _M3], fp8, tag="x")
            nc.sync.dma_start(out=x8t[:], in_=x8_d[:])
            pairs = []
            for q in range(7):
                xq = xp.tile([128, 2, _BL, _M3], bf16, tag="x")
                dma_eng = nc.scalar if q % 2 == 0 else nc.sync
                dma_eng.dma_start(out=xq[:], in_=x_d[:, 2 * q : 2 * q + 2])
                pairs.append(xq)
            x17 = xp.tile([128, 1, _BL, _M3], bf16, tag="x")
            nc.scalar.dma_start(out=x17[:], in_=x_d[:, 14:15])
            # needed only by the tail projection — after the x stream,
            # on the lighter ring
            f3t2 = consts.tile([128, _C], bf16)
            nc.sync.dma_start(out=f3t2[:], in_=f3t2_d[:])

            # touch the ACT Identity table so the tail bias-add doesn't
            # pay the on-demand ACT_TABLE_LOAD (~1.3us)
            warm = consts.tile([1, 1], f32)
            nc.scalar.add(warm[:], blob[:1, 48:49], 0.0)

            # ---- KR product on DVE, directly in ij-partition layout.
            # f1's replication pattern has period 3 in m, so three muls
            # with strided chunk views cover all 18 chunks ----
            kr = consts.tile([128, _NCH, _R], bf16)
            for c in range(3):
                nc.vector.tensor_mul(
                    kr[:, c::3, :],
                    f0r[:, c::3, :],
                    _bcast(f1r[:, c, :], bass, (128, 6, _R)),
                )

            # f2*weight on all 128 partitions (both parity halves), then
            # pre-tiled to the full [128, b*k] shape (off the critical
            # path) so the tail multiply is a plain 2D elementwise op
            f2w2 = consts.tile([128, _M3], f32)
            nc.vector.tensor_scalar_mul(f2w2[:], blob[:, 0:_M3], blob[:, 48:49])
            f2w2r = consts.tile([128, _BL, _M3], bf16)
            with nc.allow_low_precision(reason="bf16 stream for tail mul"):
                nc.vector.tensor_copy(
                    f2w2r[:], _bcast(f2w2[:], bass, (128, _BL, _M3))
                )

            # ---- main contraction: one fp32 PSUM accumulator; even
            # chunks -> partitions 0:64, odd chunks -> 64:128 ----
            z = pz.tile([128, _KB], f32, tag="z")
            chunk_of = [(x8t, m) for m in range(3)]
            for m in range(3, 17):
                chunk_of.append((pairs[(m - 3) // 2], (m - 3) % 2))
            chunk_of.append((x17, 0))
            for m in range(_NCH):
                xq, e = chunk_of[m]
                off = (m % 2) * _R
                xm_f = xq[:, e].rearrange("p b k -> p (b k)")
                for s in range(_KB // _SL):
                    nc.tensor.matmul(
                        z[off : off + _R, s * _SL : (s + 1) * _SL],
                        lhsT=kr[:, m, :],
                        rhs=xm_f[:, s * _SL : (s + 1) * _SL],
                        start=(m < 2),
                        stop=(m >= _NCH - 2),
                    )

            # ---- k-contraction at full 128-lane width, split between
            # the vector and gpsimd engines (~245 vs ~153 G elem/s) so
            # the two halves overlap; bf16 intermediates ----
            t2b = work.tile([128, _BL], bf16, tag="t2b")
            zf = work.tile([128, _BL, _M3], bf16, tag="zf")
            zf2 = zf[:].rearrange("r b k -> r (b k)")
            with nc.allow_low_precision(reason="bf16 t for PE projection"):
                nc.vector.tensor_mul(
                    zf2, z[:], f2w2r[:].rearrange("r b k -> r (b k)")
                )
                nc.vector.reduce_sum(
                    t2b[:], zf[:], axis=mybir.AxisListType.X
                )

            # keep the PE's HAM clock warm through the k-contraction so
            # the projection matmuls run at 2.4 GHz (scratch PSUM bank)
            pwarm = pz.tile([128, _SL], f32, tag="pwarm")
            for _ in range(8):
                nc.tensor.matmul(pwarm[:_R, :], lhsT=kr[:, 0, :],
                                 rhs=f3t2[:, :_SL], start=True, stop=True)

            # ---- class projection + bias; the two halves' bias-adds run
            # on different engines so they overlap ----
            op0 = po.tile([_BL, _CH], f32, tag="op0")
            op1 = po.tile([_BL, _CH], f32, tag="op1")
            osb = work.tile([_BL, _C], f32, tag="osb")
            bsb = blob[:_BL, 49:50]
            nc.tensor.matmul(op0[:], lhsT=t2b[:], rhs=f3t2[:, :_CH],
                             start=True, stop=True)
            nc.scalar.add(osb[:, :_CH], op0[:], bsb)
            nc.sync.dma_start(out=out_d[:, :_CH], in_=osb[:, :_CH])
            nc.tensor.matmul(op1[:], lhsT=t2b[:], rhs=f3t2[:, _CH:],
                             start=True, stop=True)
            nc.vector.tensor_scalar_add(osb[:, _CH:], op1[:], bsb)
            nc.scalar.dma_start(out=out_d[:, _CH:], in_=osb[:, _CH:])

    _split_excess_waits(nc, mybir)
    return nc


def _get_program():
    if "nc" not in _cache:
        _cache["nc"] = _build_program()
    return _cache["nc"]


def _host_prep(x, weight, f0, f1, f2, f3, bias):
    """Shard x over cores (batch dim) in a DMA-friendly bf16 layout;
    replicate the tiny KR factors to the ij-partition layout (pure
    gather/cast); pack f2/w/bias into one f32 blob."""
    import ml_dtypes

    bf16 = ml_dtypes.bfloat16
    x = np.asarray(x, dtype=np.float32)
    f0 = np.asarray(f0, np.float32)
    f1 = np.asarray(f1, np.float32)

    # f0r[p, m, :] = f0[(128m + p) // 48]; f1r[p, c, :] = f1[(32c + p) % 48]
    ij = (128 * np.arange(_NCH)[None, :] + np.arange(128)[:, None])
    f0r = np.ascontiguousarray(f0[ij // _M2].astype(bf16))
    jj = (32 * np.arange(3)[None, :] + np.arange(128)[:, None]) % _M2
    f1r = np.ascontiguousarray(f1[jj].astype(bf16))

    blob = np.empty((_R, 50), np.float32)
    blob[:, 0:_M3] = np.asarray(f2, np.float32).T
    blob[:, 48] = np.asarray(weight, np.float32)
    blob[:, 49] = float(np.asarray(bias, np.float32).reshape(-1)[0])
    f3t = np.asarray(f3, np.float32).T.astype(bf16)
    f3t2 = np.ascontiguousarray(np.concatenate([f3t, f3t], axis=0))
    in_maps = []
    fp8 = ml_dtypes.float8_e4m3
    for c in range(_NCORES):
        xc = x[c * _BL : (c + 1) * _BL]
        # [b, ij, k] -> [p, m, b, k] with ij = 128*m + p
        xp_ = xc.reshape(_BL, _NCH, 128, _M3).transpose(2, 1, 0, 3)
        x8 = np.ascontiguousarray(xp_[:, :3].astype(fp8))
        xd = np.ascontiguousarray(xp_[:, 3:].astype(bf16))
        in_maps.append(
            {"x8": x8, "x": xd, "f0r": f0r, "f1r": f1r, "blob": blob,
             "f3t2": f3t2}
        )
    return in_maps


LAST_EXEC_NS = None


def kernel(x, weight, f0, f1, f2, f3, bias):
    global LAST_EXEC_NS
    from concourse.bass_utils import run_bass_kernel_spmd

    nc = _get_program()
    in_maps = _host_prep(x, weight, f0, f1, f2, f3, bias)
    trace = bool(int(os.environ.get("BASS_KERNEL_TRACE", "0")))
    res = run_bass_kernel_spmd(nc, in_maps, list(range(_NCORES)), trace=trace)
    LAST_EXEC_NS = res.exec_time_ns
    out = np.concatenate([res.results[c]["out"] for c in range(_NCORES)], axis=0)
    return np.ascontiguousarray(out.astype(np.float32, copy=False))


# revision 24
# speedup vs baseline: 1.0264x; 1.0264x over previous
"""CP tensor-regression-layer kernel for Trainium2 (8 NeuronCores).

Computation (matches the reference einsum pair):
    t[b, r]  = sum_{i,j,k} x[b,i,j,k] * f0[i,r] * f1[j,r] * f2[k,r]
    out[b,c] = sum_r t[b,r] * weight[r] * f3[c,r] + bias[0]

Strategy: data-parallel over the batch dim (32 batches per core, CP
factors replicated).  Per core the big contraction is restructured as
    z[r, b, k] = sum_{ij} (f0[i,r]*f1[j,r]) * x[b, ij, k]
a K=2304 matmul against the Khatri-Rao product of f0 and f1, run as 18
K-chunks of 128 partitions.  x and the KR factors stream as bf16 (the
dominant HBM traffic halves vs f32; measured rel-err ~4e-3 vs the 2e-2
gate), accumulating fp32 in PSUM.  Even chunks write PSUM partitions
0:64, odd chunks 64:128, so the k-contraction against f2*weight runs on
the vector engine at the full 128-lane width, and the class projection
contracts all 128 partitions against a host-stacked [f3; f3] so the
parity halves sum for free.

The KR factors arrive pre-replicated to the ij-partition layout (f0
rows repeated to [128, 18, 64]; f1's pattern has period 3 in the chunk
index so [128, 3, 64] suffices), which removes the PE-transpose +
cast pipeline from the critical path: the device still computes the KR
product, both contractions, the weighting and bias.

The Tile epilogue is replaced by a lighter one: the standard drain +
barrier, then semaphore clears distributed across all five engines
(instead of one serial gpsimd loop plus a multi-microsecond DMA-reset
round trip), then a final barrier.  Re-run safety (sems back at zero)
is preserved; verified by back-to-back executions.
"""

import os

import numpy as np

_B, _M1, _M2, _M3, _C, _R = 256, 48, 48, 48, 1000, 64
_NCORES = 8
_BL = _B // _NCORES          # 32 batches per core
_IJ = _M1 * _M2              # 2304 contraction size (i,j fused)
_NCH = _IJ // 128            # 18 K-chunks of 128 partitions
_KB = _BL * _M3              # 1536 moving columns (b,k fused)
_SL = 512                    # matmul slice width (one PSUM bank, fp32)
_CH = _C // 2                # class-projection column half

_cache = {}


def _split_excess_waits(nc, mybir, max_waits=1):
    """Walrus in this container rejects >1 sync-wait per instruction
    ("Too many sync wait commands").  Move excess waits onto chained
    NoOps inserted just before the offending instruction (same engine,
    so program order preserves the gating)."""
    for bb in nc.m.functions[0].blocks:
        insts = bb.instructions
        i = 0
        while i < len(insts):
            inst = insts[i]
            si = getattr(inst, "sync_info", None)
            waits = list(si.on_wait) if si is not None and si.on_wait else []
            if len(waits) > max_waits:
                rest, keep = waits[:-max_waits], waits[-max_waits:]
                pos = i
                for j in range(0, len(rest), max_waits):
                    nop = mybir.InstNoOp(
                        name=f"I-waitsplit-{nc.next_id()}",
                        engine=inst.engine,
                        ins=[],
                        outs=[],
                        sync_info=mybir.SyncInfo(
                            on_wait=list(rest[j : j + max_waits]), on_update=[]
                        ),
                    )
                    nc.register_instruction(nop)
                    insts.insert(pos, nop)
                    pos += 1
                    i += 1
                si.on_wait = keep
            i += 1


def _bcast(ap, bass, shape3):
    """AP broadcast helper: make a 3D view with a stride-0 middle dim."""
    try:
        return ap.unsqueeze(1).broadcast_to(shape3)
    except Exception:
        a = ap.ap
        return bass.AP(
            tensor=ap.tensor,
            offset=ap.offset,
            ap=[list(a[0]), [0, shape3[1]], list(a[1])],
        )


def _patch_light_teardown(tile):
    """Replace TileContext._drain_and_barrier with an equivalent but
    cheaper epilogue: same final-value drain + barrier, but semaphore
    clears spread across all engines and no gpsimd dma_reset round trip
    (every DMA's completion is already waited on by the drain)."""
    if getattr(tile.TileContext, "_light_teardown", False):
        return
    orig = tile.TileContext._drain_and_barrier

    def _light(self, tick_clock, wait_clock):
        nc = self.nc
        # probe the internals the light path relies on; any mismatch
        # (version skew) falls back to the stock teardown
        ok = (
            hasattr(tile, "ScopedClock")
            and hasattr(nc, "_tile_sem_poison_stack")
            and hasattr(self, "sems")
            and self.sems is not None
            and hasattr(self, "_sem_poison")
            and hasattr(nc._state, "prepend_free_semaphores")
            and all(
                hasattr(e, "sem_clear")
                for e in (nc.sync, nc.scalar, nc.vector, nc.tensor, nc.gpsimd)
            )
        )
        if not ok:
            return orig(self, tick_clock, wait_clock)
        drain_inst = nc.sync.drain()
        wait_clock.add_sem_waits(
            drain_inst.ins,
            tile.ScopedClock({None: tick_clock.global_clock}),
        )
        nc.all_engine_barrier()
        popped = nc._tile_sem_poison_stack.pop()
        assert popped is self._sem_poison
        handles = list(self.sems.allocated().values())
        engines = [nc.sync, nc.scalar, nc.vector, nc.tensor, nc.gpsimd]
        for i, h in enumerate(handles):
            engines[i % len(engines)].sem_clear(h)
        sem_nums = [h.num if hasattr(h, "num") else int(h) for h in handles]
        nc._state.prepend_free_semaphores(sem_nums)
        for poison_set in nc._tile_sem_poison_stack:
            poison_set.update(sem_nums)
        nc.all_engine_barrier()

    tile.TileContext._drain_and_barrier = _light
    tile.TileContext._light_teardown = True
    tile.TileContext._orig_drain_and_barrier = orig


def _build_program():
    import ml_dtypes
    import concourse.bass as bass
    import concourse.tile as tile
    from concourse import mybir

    f32 = mybir.dt.float32
    bf16 = mybir.dt.bfloat16

    _patch_light_teardown(tile)

    nc = bass.Bass("TRN2", target_bir_lowering=False, debug=False,
                   num_devices=_NCORES)

    # x pairs: chunk m = 2q+e lives at x_d[:, q, e]; last two chunks are
    # shipped separately so the final arrival is small.
    x_d = nc.dram_tensor("x", [128, _NCH, _BL, _M3], bf16,
                         kind="ExternalInput")
    f0r_d = nc.dram_tensor("f0r", [128, _NCH, _R], bf16, kind="ExternalInput")
    f1r_d = nc.dram_tensor("f1r", [128, 3, _R], bf16, kind="ExternalInput")
    # blob columns: 0:48 f2^T | 48 w | 49 bias
    blob_d = nc.dram_tensor("blob", [_R, 50], f32, kind="ExternalInput")
    f3t2_d = nc.dram_tensor("f3t2", [128, _C], bf16, kind="ExternalInput")
    out_d = nc.dram_tensor("out", [_BL, _C], f32, kind="ExternalOutput")

    with tile.TileContext(nc) as tc:
        with (
            tc.tile_pool(name="consts", bufs=1) as consts,
            tc.tile_pool(name="xp", bufs=10) as xp,
            tc.tile_pool(name="work", bufs=1) as work,
            tc.tile_pool(name="pz", bufs=1, space=bass.MemorySpace.PSUM) as pz,
            tc.tile_pool(name="po", bufs=1, space=bass.MemorySpace.PSUM) as po,
        ):
            # ---- DMA issue order drives everything: factors first (they
            # gate the first matmul), then the x stream split across the
            # two HWDGE rings, pairs 0-7 then chunks 16/17 singly ----
            f0r = consts.tile([128, _NCH, _R], bf16)
            nc.sync.dma_start(out=f0r[:], in_=f0r_d[:])
            f1r = consts.tile([128, 3, _R], bf16)
            nc.scalar.dma_start(out=f1r[:], in_=f1r_d[:])
            blob = consts.tile([128, 50], f32)
            nc.sync.dma_start(out=blob[:_R, :], in_=blob_d[:])
            nc.scalar.dma_start(out=blob[_R:, :], in_=blob_d[:])

            xts = []
            for q in range(8):
                xq = xp.tile([128, 2, _BL, _M3], bf16, tag="x")
                dma_eng = nc.sync if q % 2 == 0 else nc.scalar
                dma_eng.dma_start(out=xq[:], in_=x_d[:, 2 * q : 2 * q + 2])
                xts.append((xq, 2))
            for m in (16, 17):
                xs = xp.tile([128, 1, _BL, _M3], bf16, tag="x")
                dma_eng = nc.sync if m == 16 else nc.scalar
                dma_eng.dma_start(out=xs[:], in_=x_d[:, m : m + 1])
                xts.append((xs, 1))
            # needed only by the tail projection — after the x stream
            f3t2 = consts.tile([128, _C], bf16)
            nc.scalar.dma_start(out=f3t2[:], in_=f3t2_d[:])

            # touch the ACT Identity table so the tail bias-add doesn't
            # pay the on-demand ACT_TABLE_LOAD (~1.3us)
            warm = consts.tile([1, 1], f32)
            nc.scalar.add(warm[:], blob[:1, 48:49], 0.0)

            # ---- KR product on DVE, directly in ij-partition layout.
            # f1's replication pattern has period 3 in m, so three muls
            # with strided chunk views cover all 18 chunks ----
            kr = consts.tile([128, _NCH, _R], bf16)
            for c in range(3):
                nc.vector.tensor_mul(
                    kr[:, c::3, :],
                    f0r[:, c::3, :],
                    _bcast(f1r[:, c, :], bass, (128, 6, _R)),
                )

            # f2*weight on all 128 partitions (both parity halves), then
            # pre-tiled to the full [128, b*k] shape (off the critical
            # path) so the tail multiply is a plain 2D elementwise op
            f2w2 = consts.tile([128, _M3], f32)
            nc.vector.tensor_scalar_mul(f2w2[:], blob[:, 0:_M3], blob[:, 48:49])
            f2w2r = consts.tile([128, _BL, _M3], bf16)
            with nc.allow_low_precision(reason="bf16 stream for tail mul"):
                nc.vector.tensor_copy(
                    f2w2r[:], _bcast(f2w2[:], bass, (128, _BL, _M3))
                )

            # ---- main contraction: one fp32 PSUM accumulator; even
            # chunks -> partitions 0:64, odd chunks -> 64:128 ----
            z = pz.tile([128, _KB], f32, tag="z")
            chunk_of = []
            for xq, n in xts:
                base = len(chunk_of)
                for e in range(n):
                    chunk_of.append((xq, e))
            for m in range(_NCH):
                xq, e = chunk_of[m]
                off = (m % 2) * _R
                xm_f = xq[:, e].rearrange("p b k -> p (b k)")
                for s in range(_KB // _SL):
                    nc.tensor.matmul(
                        z[off : off + _R, s * _SL : (s + 1) * _SL],
                        lhsT=kr[:, m, :],
                        rhs=xm_f[:, s * _SL : (s + 1) * _SL],
                        start=(m < 2),
                        stop=(m >= _NCH - 2),
                    )

            # ---- k-contraction at full 128-lane width, split between
            # the vector and gpsimd engines (~245 vs ~153 G elem/s) so
            # the two halves overlap; bf16 intermediates ----
            t2b = work.tile([128, _BL], bf16, tag="t2b")
            zf = work.tile([128, _BL, _M3], bf16, tag="zf")
            zf2 = zf[:].rearrange("r b k -> r (b k)")
            with nc.allow_low_precision(reason="bf16 t for PE projection"):
                nc.vector.tensor_mul(
                    zf2, z[:], f2w2r[:].rearrange("r b k -> r (b k)")
                )
                nc.vector.reduce_sum(
                    t2b[:], zf[:], axis=mybir.AxisListType.X
                )

            # keep the PE's HAM clock warm through the k-contraction so
            # the projection matmuls run at 2.4 GHz (scratch PSUM bank)
            pwarm = pz.tile([128, _SL], f32, tag="pwarm")
            for _ in range(8):
                nc.tensor.matmul(pwarm[:_R, :], lhsT=kr[:, 0, :],
                                 rhs=f3t2[:, :_SL], start=True, stop=True)

            # ---- class projection + bias; the two halves' bias-adds run
            # on different engines so they overlap ----
            op0 = po.tile([_BL, _CH], f32, tag="op0")
            op1 = po.tile([_BL, _CH], f32, tag="op1")
            osb = work.tile([_BL, _C], f32, tag="osb")
            bsb = blob[:_BL, 49:50]
            nc.tensor.matmul(op0[:], lhsT=t2b[:], rhs=f3t2[:, :_CH],
                             start=True, stop=True)
            nc.scalar.add(osb[:, :_CH], op0[:], bsb)
            nc.sync.dma_start(out=out_d[:, :_CH], in_=osb[:, :_CH])
            nc.tensor.matmul(op1[:], lhsT=t2b[:], rhs=f3t2[:, _CH:],
                             start=True, stop=True)
            nc.vector.tensor_scalar_add(osb[:, _CH:], op1[:], bsb)
            nc.scalar.dma_start(out=out_d[:, _CH:], in_=osb[:, _CH:])

    _split_excess_waits(nc, mybir)
    return nc


def _get_program():
    if "nc" not in _cache:
        _cache["nc"] = _build_program()
    return _cache["nc"]


def _host_prep(x, weight, f0, f1, f2, f3, bias):
    """Shard x over cores (batch dim) in a DMA-friendly bf16 layout;
    replicate the tiny KR factors to the ij-partition layout (pure
    gather/cast); pack f2/w/bias into one f32 blob."""
    import ml_dtypes

    bf16 = ml_dtypes.bfloat16
    x = np.asarray(x, dtype=np.float32)
    f0 = np.asarray(f0, np.float32)
    f1 = np.asarray(f1, np.float32)

    # f0r[p, m, :] = f0[(128m + p) // 48]; f1r[p, c, :] = f1[(32c + p) % 48]
    ij = (128 * np.arange(_NCH)[None, :] + np.arange(128)[:, None])
    f0r = np.ascontiguousarray(f0[ij // _M2].astype(bf16))
    jj = (32 * np.arange(3)[None, :] + np.arange(128)[:, None]) % _M2
    f1r = np.ascontiguousarray(f1[jj].astype(bf16))

    blob = np.empty((_R, 50), np.float32)
    blob[:, 0:_M3] = np.asarray(f2, np.float32).T
    blob[:, 48] = np.asarray(weight, np.float32)
    blob[:, 49] = float(np.asarray(bias, np.float32).reshape(-1)[0])
    f3t = np.asarray(f3, np.float32).T.astype(bf16)
    f3t2 = np.ascontiguousarray(np.concatenate([f3t, f3t], axis=0))
    in_maps = []
    for c in range(_NCORES):
        xc = x[c * _BL : (c + 1) * _BL]
        # [b, ij, k] -> [p, m, b, k] with ij = 128*m + p
        xd = np.ascontiguousarray(
            xc.reshape(_BL, _NCH, 128, _M3).transpose(2, 1, 0, 3).astype(bf16)
        )
        in_maps.append(
            {"x": xd, "f0r": f0r, "f1r": f1r, "blob": blob, "f3t2": f3t2}
        )
    return in_maps


LAST_EXEC_NS = None


def kernel(x, weight, f0, f1, f2, f3, bias):
    global LAST_EXEC_NS
    from concourse.bass_utils import run_bass_kernel_spmd

    nc = _get_program()
    in_maps = _host_prep(x, weight, f0, f1, f2, f3, bias)
    trace = bool(int(os.environ.get("BASS_KERNEL_TRACE", "0")))
    res = run_bass_kernel_spmd(nc, in_maps, list(range(_NCORES)), trace=trace)
    LAST_EXEC_NS = res.exec_time_ns
    out = np.concatenate([res.results[c]["out"] for c in range(_NCORES)], axis=0)
    return np.ascontiguousarray(out.astype(np.float32, copy=False))
